# revision 51
# baseline (speedup 1.0000x reference)
"""Trainium2 Bass kernel for nn_MultiHeadDenseDotProductAttentionLayer.

Sharding: one attention head per NeuronCore (8 heads / 8 cores), per the
tensor-parallel hint.  Each core computes its head's Q/K projections from the
384-row slab of x that the reference's raw-view reshape maps to that head,
the V projection over all rows for its 64 weight columns, the [3072, 3072]
attention (scores computed transposed so the softmax denominator folds into
the A@V matmul as a ones-column), and writes its [3072, 64] output slice.

Host side only reshapes/slices/transposes numpy inputs to build per-core
input maps and concatenates the per-core output slices.
"""

import os
import sys

import numpy as np

for _p in ("/opt/trn_rl_repo", "/root/.axon_site/_ro/trn_rl_repo"):
    if os.path.isdir(_p) and _p not in sys.path:
        sys.path.insert(0, _p)

import concourse.bass as bass
import concourse.tile as tile
from concourse import bacc, mybir
from concourse.masks import make_identity

N = 3072
IN_DIM = 512
H = 8
D = 64
A = 8
HD = H * D          # 512
SLAB = N // H       # 384
NCORES = 8
KC = IN_DIM // 128  # 4 contraction chunks
RT = SLAB // 128    # 3 row tiles per slab
MT = N // 128       # 24 m-chunks
IT = N // 512       # 6 i-chunks
CLAMP_RAW = 40.0    # clip(score, 5) with score = raw/8  ->  min(raw, 40)
FP = mybir.dt.float32
FPR = mybir.dt.float32r


def _r(ap):
    """Reinterpret an fp32 AP as float32r: full-rate PE streaming for fp32
    data (vs 4 cycles/row for plain fp32 matmuls)."""
    return ap.bitcast(FPR)

# dtype for the clipped scores / exponentiated attention weights and the V
# operand of the second matmul.  float16 keeps ~5e-4 relative accuracy while
# enabling the 16-bit 2x packed read mode on the ACT exp pass.
E_DT = mybir.dt.float16
AF = mybir.ActivationFunctionType


def _build(has_bq, has_bk, has_bv):
    nc = bacc.Bacc()

    xT = nc.declare_dram_parameter("xT", [IN_DIM, N], E_DT, False)
    xsT = nc.declare_dram_parameter("xsT", [IN_DIM, SLAB], FPR, False)
    wq_d = nc.declare_dram_parameter("wq", [IN_DIM, HD], FPR, False)
    wk_d = nc.declare_dram_parameter("wk", [IN_DIM, HD], FPR, False)
    wv_d = nc.declare_dram_parameter("wv", [IN_DIM, D], E_DT, False)
    angT_d = nc.declare_dram_parameter("angT", [A, SLAB], FP, False)
    s_d = nc.declare_dram_parameter("S", [A, HD // 2], FP, False)
    if has_bq:
        bq_d = nc.declare_dram_parameter("bq", [1, HD], FP, False)
    if has_bk:
        bk_d = nc.declare_dram_parameter("bk", [1, HD], FP, False)
    if has_bv:
        bv_d = nc.declare_dram_parameter("bv", [1, D], FP, False)
    out_d = nc.declare_dram_parameter("out", [N, D], FP, True)

    with tile.TileContext(nc) as tc:
        with (
            tc.tile_pool(name="consts", bufs=1) as consts,
            tc.tile_pool(name="dram", bufs=1, space="DRAM") as dram,
        ):
            ident = consts.tile([128, 128], FP)
            make_identity(nc, ident)
            identr = consts.tile([128, 128], FPR)
            nc.vector.tensor_copy(identr, ident)
            halfpi = consts.tile([128, 1], FP)
            nc.vector.memset(halfpi, float(np.pi / 2))

            # ---- constant loads (SP queue order == criticality) ------
            angT_sb = consts.tile([A, SLAB], FP)
            nc.sync.dma_start(out=angT_sb, in_=angT_d[:, :])
            s_sb = consts.tile([A, HD // 2], FP)
            nc.sync.dma_start(out=s_sb, in_=s_d[:, :])
            xsT_sb = consts.tile([128, KC, SLAB], FPR)
            nc.sync.dma_start(
                out=xsT_sb, in_=xsT.rearrange("(kc p) r -> p kc r", p=128)
            )
            wk_sb = consts.tile([128, KC, HD], FPR)
            nc.sync.dma_start(
                out=wk_sb, in_=wk_d.rearrange("(kc p) c -> p kc c", p=128)
            )
            wv_sb = consts.tile([128, KC, D], E_DT)
            nc.sync.dma_start(
                out=wv_sb, in_=wv_d.rearrange("(kc p) c -> p kc c", p=128)
            )
            wq_sb = consts.tile([128, KC, HD], FPR)
            nc.sync.dma_start(
                out=wq_sb, in_=wq_d.rearrange("(kc p) c -> p kc c", p=128)
            )
            if has_bq:
                bq_sb = consts.tile([1, HD], FP)
                nc.sync.dma_start(out=bq_sb, in_=bq_d[:, :])
            if has_bk:
                bk_sb = consts.tile([1, HD], FP)
                nc.sync.dma_start(out=bk_sb, in_=bk_d[:, :])
            if has_bv:
                bv_sb = consts.tile([1, D], FP)
                nc.sync.dma_start(out=bv_sb, in_=bv_d[:, :])
            if has_bq or has_bk or has_bv:
                ones_col = consts.tile([1, 128], FP)
                nc.vector.memset(ones_col, 1.0)

            # full x^T on the SWDGE queue (keeps the SP queue free for the
            # small latency-critical loads), split so the V projection can
            # start before the whole 6 MB lands
            xT_sb = consts.tile([128, KC, N], E_DT)
            for q in range(4):
                sl = slice(q * (N // 4), (q + 1) * (N // 4))
                nc.gpsimd.dma_start(
                    out=xT_sb[:, :, sl],
                    in_=xT[:, sl].rearrange("(kc p) m -> p kc m", p=128),
                )

            # persistent operands of the attention loop
            qdT = consts.tile([D, N], FPR)           # Q.reshape(3072,64).T
            ks_sb = consts.tile([D, N], FPR)         # K raw-view [64, 3072]
            v_sb = consts.tile([128, MT, D + 1], E_DT)  # [V | 1] per m-chunk
            k_scr = dram.tile([SLAB, HD], FPR)

            # ================= prologue ===============================
            with (
                tc.tile_pool(name="small", bufs=1) as small,
                tc.tile_pool(name="ppsum", bufs=2, space="PSUM") as ppsum,
                tc.tile_pool(name="trig", bufs=3) as trig,
                tc.tile_pool(name="qk", bufs=3) as qk,
            ):
                # softmax(S, axis=1) -> P, then column-doubled P_rep
                smax = small.tile([A, 1], FP)
                nc.vector.tensor_reduce(
                    out=smax, in_=s_sb, axis=mybir.AxisListType.X,
                    op=mybir.AluOpType.max,
                )
                negmax = small.tile([A, 1], FP)
                nc.vector.tensor_scalar_mul(negmax, smax, -1.0)
                p_sb = small.tile([A, HD // 2], FP)
                psum_acc = small.tile([A, 1], FP)
                nc.scalar.activation(
                    p_sb, s_sb, AF.Exp, bias=negmax, scale=1.0,
                    accum_out=psum_acc,
                )
                rec8 = small.tile([A, 1], FP)
                nc.vector.reciprocal(rec8, psum_acc)
                p2_sb = small.tile([A, HD // 2], FP)
                nc.vector.tensor_scalar_mul(p2_sb, p_sb, rec8)
                p_rep = small.tile([A, HD], FP)
                pr3 = p_rep.rearrange("a (c two) -> a c two", two=2)
                nc.scalar.copy(pr3[:, :, 0], p2_sb)
                nc.scalar.copy(pr3[:, :, 1], p2_sb)

                # rope combine helper: r = x*cos + shuffle(x)*sin_pm
                def rope(pr_ps, cos_t, spm, spm4):
                    r_t = qk.tile([128, HD], FPR, tag="rt", name="r_t")
                    nc.vector.tensor_tensor(
                        r_t, pr_ps, cos_t, mybir.AluOpType.mult
                    )
                    tmp = qk.tile([128, HD], FP, tag="tmp", name="tmp")
                    tmp4 = tmp.rearrange("p (cb h t) -> p cb h t", cb=8, h=2)
                    x4 = pr_ps.rearrange(
                        "p (cb t two) -> p cb t two", cb=8, two=2
                    )
                    nc.vector.tensor_tensor(
                        tmp4[:, :, 0, :], x4[:, :, :, 1], spm4[:, :, 0, :],
                        mybir.AluOpType.mult,
                    )
                    nc.vector.tensor_tensor(
                        tmp4[:, :, 1, :], x4[:, :, :, 0], spm4[:, :, 1, :],
                        mybir.AluOpType.mult,
                    )
                    nc.vector.tensor_tensor(r_t, r_t, tmp, mybir.AluOpType.add)
                    return r_t

                def proj(w_sb, b_sb, rsl):
                    pr_ps = ppsum.tile([128, HD], FP, tag="proj", name="pr", bufs=3)
                    if b_sb is not None:
                        nc.tensor.matmul(
                            pr_ps, ones_col, b_sb, start=True, stop=False
                        )
                    for kc in range(KC):
                        nc.tensor.matmul(
                            pr_ps,
                            xsT_sb[:, kc, rsl],
                            w_sb[:, kc, :],
                            start=(kc == 0 and b_sb is None),
                            stop=(kc == KC - 1),
                        )
                    return pr_ps

                # K pass first (plus theta -> cos/sin): the Ks round trip
                # gates the whole attention loop
                trigs = []
                for rt in range(RT):
                    rsl = slice(rt * 128, (rt + 1) * 128)
                    th_ps = ppsum.tile([128, HD], FP, tag="th")
                    nc.tensor.matmul(
                        th_ps, angT_sb[:, rsl], p_rep, start=True, stop=True
                    )
                    cos_t = trig.tile([128, HD], FP, tag="cos", name="cos_t")
                    nc.scalar.activation(cos_t, th_ps, AF.Sin, bias=halfpi)
                    # sin with the rotate-half sign pattern folded in:
                    # first 32 of each 64-block negative, last 32 positive
                    spm = trig.tile([128, HD], FP, tag="spm", name="spm")
                    spm4 = spm.rearrange("p (cb h t) -> p cb h t", cb=8, h=2)
                    thv = th_ps.rearrange("p (cb t) -> p cb t", cb=8)
                    nc.scalar.activation(
                        spm4[:, :, 0, :], thv[:, :, 0:32], AF.Sin, scale=-1.0
                    )
                    nc.scalar.activation(
                        spm4[:, :, 1, :], thv[:, :, 32:64], AF.Sin, scale=1.0
                    )
                    trigs.append((cos_t, spm, spm4))

                    pr_ps = proj(wk_sb, bk_sb if has_bk else None, rsl)
                    r_t = rope(pr_ps, cos_t, spm, spm4)
                    nc.sync.dma_start(out=k_scr[rsl, :], in_=r_t)

                # K raw view: row j of [64, 3072] = rows 6j..6j+6 of [384, 512]
                ks_v = k_scr.rearrange("(j rr) c -> j (rr c)", j=D)
                nc.sync.dma_start(
                    out=ks_sb[:, 0:N // 2], in_=ks_v[:, 0:N // 2]
                )
                nc.scalar.dma_start(
                    out=ks_sb[:, N // 2:N], in_=ks_v[:, N // 2:N]
                )

                # Q pass; qdT[j, rt*1024 + rr*8 + cb] = r_t[rr, 64cb + j]
                qdT_v = qdT.rearrange(
                    "j (rt rr cb) -> j rt rr cb", rt=RT, cb=8
                )
                for rt in range(RT):
                    rsl = slice(rt * 128, (rt + 1) * 128)
                    cos_t, spm, spm4 = trigs[rt]
                    pr_ps = proj(wq_sb, bq_sb if has_bq else None, rsl)
                    r_t = rope(pr_ps, cos_t, spm, spm4)
                    for cb in range(8):
                        tr_ps = ppsum.tile([D, 128], FPR, tag="tr")
                        nc.tensor.transpose(
                            tr_ps, r_t[:, cb * D:(cb + 1) * D], identr
                        )
                        nc.vector.tensor_copy(qdT_v[:, rt, :, cb], tr_ps)



            # ================= attention main loop ====================
            with (
                tc.tile_pool(name="opsum", bufs=1, space="PSUM") as opsum,
                tc.tile_pool(name="stpsum", bufs=2, space="PSUM") as stp,
                tc.tile_pool(name="ets", bufs=5) as ets,
                tc.tile_pool(name="fin", bufs=2) as fin,
                tc.tile_pool(name="outp", bufs=2) as outp,
            ):
                o_tiles = [
                    opsum.tile([D + 1, 512], FP, tag=f"o{i}", name=f"o{i}")
                    for i in range(IT)
                ]
                for mt in range(MT):
                    # V projection for this m-chunk (natural layout + ones
                    # column); interleaved here so prologue PE stays short
                    msl = slice(mt * 128, (mt + 1) * 128)
                    v_ps = stp.tile([128, D], FP, tag="st", name="v_ps")
                    if has_bv:
                        nc.tensor.matmul(
                            v_ps, ones_col, bv_sb, start=True, stop=False
                        )
                    for kc in range(KC):
                        nc.tensor.matmul(
                            v_ps,
                            xT_sb[:, kc, msl],
                            wv_sb[:, kc, :],
                            start=(kc == 0 and not has_bv),
                            stop=(kc == KC - 1),
                        )
                    nc.vector.tensor_copy(v_sb[:, mt, 0:D], v_ps)
                    nc.gpsimd.memset(v_sb[:, mt, D:D + 1], 1.0)

                    ks_l = ks_sb[:, mt * 128:(mt + 1) * 128]
                    v_l = v_sb[:, mt, :]
                    for it in range(IT):
                        st = stp.tile([128, 512], FP, tag="st")
                        nc.tensor.matmul(
                            st, ks_l, qdT[:, it * 512:(it + 1) * 512],
                            start=True, stop=True,
                        )
                        etc = ets.tile([128, 512], E_DT, tag="etc")
                        nc.vector.tensor_scalar_min(etc, st, CLAMP_RAW)
                        ete = ets.tile([128, 512], E_DT, tag="ete")
                        nc.scalar.activation(ete, etc, AF.Exp, scale=0.125)
                        nc.tensor.matmul(
                            o_tiles[it], v_l, ete,
                            start=(mt == 0), stop=(mt == MT - 1),
                            skip_group_check=True,
                        )

                # normalize + transpose back to [i, d], write out
                for it in range(IT):
                    ot = fin.tile([D + 1, 512], FP, tag="ot")
                    nc.scalar.copy(ot, o_tiles[it])
                    ob = outp.tile([128, 4, D], FP, tag="ob")
                    for s in range(4):
                        on_ps = stp.tile([128, D + 1], FP, tag="st")
                        nc.tensor.transpose(
                            on_ps, ot[:, s * 128:(s + 1) * 128],
                            ident[0:D + 1, 0:D + 1],
                        )
                        recd = fin.tile([128, 1], FP, tag="recd")
                        nc.vector.reciprocal(recd, on_ps[:, D:D + 1])
                        nc.vector.tensor_scalar_mul(
                            ob[:, s, :], on_ps[:, 0:D], recd
                        )
                    nc.sync.dma_start(
                        out=out_d[it * 512:(it + 1) * 512, :].rearrange(
                            "(s p) d -> p s d", p=128
                        ),
                        in_=ob,
                    )

    nc.compile()
    nc.finalize()
    return nc


_CACHE = {}


def _get_nc(has_bq, has_bk, has_bv):
    key = (has_bq, has_bk, has_bv)
    if key not in _CACHE:
        _CACHE[key] = _build(*key)
    return _CACHE[key]


def _in_maps(x, node_rotation_angles, Wq, bq, Wk, bk, Wv, bv, S):
    f32 = np.float32
    x = np.asarray(x, f32)
    ang = np.asarray(node_rotation_angles, f32)
    Wq = np.asarray(Wq, f32)
    Wk = np.asarray(Wk, f32)
    Wv = np.asarray(Wv, f32)
    S = np.asarray(S, f32)
    bq = np.asarray(bq, f32)
    bk = np.asarray(bk, f32)
    bv = np.asarray(bv, f32)

    has_bq = bool(np.any(bq))
    has_bk = bool(np.any(bk))
    has_bv = bool(np.any(bv))

    xT = np.ascontiguousarray(x.T)
    xT16 = xT.astype(np.float16)
    angT = np.ascontiguousarray(ang.T)

    maps = []
    for h in range(NCORES):
        m = {
            "xT": xT16,
            "xsT": np.ascontiguousarray(xT[:, h * SLAB:(h + 1) * SLAB]),
            "wq": Wq,
            "wk": Wk,
            "wv": np.ascontiguousarray(
                Wv[:, h * D:(h + 1) * D]
            ).astype(np.float16),
            "angT": np.ascontiguousarray(angT[:, h * SLAB:(h + 1) * SLAB]),
            "S": S,
        }
        if has_bq:
            m["bq"] = bq.reshape(1, HD)
        if has_bk:
            m["bk"] = bk.reshape(1, HD)
        if has_bv:
            m["bv"] = np.ascontiguousarray(bv[h * D:(h + 1) * D]).reshape(1, D)
        maps.append(m)
    return (has_bq, has_bk, has_bv), maps


def _assemble(results):
    out = np.empty((N, HD), np.float32)
    for h in range(NCORES):
        out[:, h * D:(h + 1) * D] = results[h]["out"]
    return out.reshape(N, H, D)


class _Runner:
    """Persistent shard_map'd executor for the SPMD bass kernel.

    Mirrors bass2jax.run_bass_via_pjrt but keeps the compiled function and
    lets inputs stay on device across calls so execution can be timed
    without per-call host transfer / dispatch rebuild cost.
    """

    def __init__(self, nc):
        import jax
        from jax.sharding import Mesh, PartitionSpec
        from jax.experimental.shard_map import shard_map

        from concourse import bass2jax, mybir as _mb

        bass2jax.install_neuronx_cc_hook()
        self.nc = nc
        partition_name = (
            nc.partition_id_tensor.name if nc.partition_id_tensor else None
        )
        in_names, out_names, out_avals, zero_outs = [], [], [], []
        for alloc in nc.m.functions[0].allocations:
            if not isinstance(alloc, _mb.MemoryLocationSet):
                continue
            name = alloc.memorylocations[0].name
            if alloc.kind == "ExternalInput":
                if name != partition_name:
                    in_names.append(name)
            elif alloc.kind == "ExternalOutput":
                out_names.append(name)
                shape = tuple(alloc.tensor_shape)
                dtype = _mb.dt.np(alloc.dtype)
                out_avals.append(jax.core.ShapedArray(shape, dtype))
                zero_outs.append(np.zeros(shape, dtype))
        self.in_names = list(in_names)
        self.out_names = out_names
        self.out_avals = out_avals
        self.zero_outs = zero_outs
        n_params = len(in_names)
        all_names = in_names + out_names
        if partition_name is not None:
            all_names = all_names + [partition_name]

        def _body(*args):
            operands = list(args)
            if partition_name is not None:
                operands.append(bass2jax.partition_id_tensor())
            outs = bass2jax._bass_exec_p.bind(
                *operands,
                out_avals=tuple(out_avals),
                in_names=tuple(all_names),
                out_names=tuple(out_names),
                lowering_input_output_aliases=(),
                sim_require_finite=True,
                sim_require_nnan=True,
                nc=nc,
            )
            return tuple(outs)

        devices = jax.devices()[:NCORES]
        self.mesh = Mesh(np.asarray(devices), ("core",))
        n_outs = len(out_names)
        self.n_params = n_params
        self.n_outs = n_outs
        in_specs = (PartitionSpec("core"),) * (n_params + n_outs)
        out_specs = (PartitionSpec("core"),) * n_outs
        self.fn = jax.jit(
            shard_map(
                _body, mesh=self.mesh, in_specs=in_specs,
                out_specs=out_specs, check_rep=False,
            ),
            donate_argnums=tuple(range(n_params, n_params + n_outs)),
            keep_unused=True,
        )
        self._body = _body
        self._shard_map = shard_map
        self._PartitionSpec = PartitionSpec
        self.jax = jax

    def build_multi(self, k):
        """jit fn executing the kernel k times back-to-back on device.

        Takes (inputs..., zeros_0..., zeros_1..., ..., zeros_{k-1}...);
        bass effects keep the k custom calls ordered, so wall-time slope
        over k measures pure on-device execution time."""
        jax = self.jax
        np_, no, body = self.n_params, self.n_outs, self._body

        def _multi(*args):
            ins = args[:np_]
            outs = None
            for i in range(k):
                z = args[np_ + i * no: np_ + (i + 1) * no]
                outs = body(*ins, *z)
            return outs

        in_specs = (self._PartitionSpec("core"),) * (np_ + k * no)
        out_specs = (self._PartitionSpec("core"),) * no
        return jax.jit(
            self._shard_map(
                _multi, mesh=self.mesh, in_specs=in_specs,
                out_specs=out_specs, check_rep=False,
            ),
            donate_argnums=tuple(range(np_, np_ + k * no)),
            keep_unused=True,
        )

    def stage_inputs(self, maps):
        from jax.sharding import NamedSharding, PartitionSpec

        sh = NamedSharding(self.mesh, PartitionSpec("core"))
        staged = []
        for i, name in enumerate(self.in_names):
            arr = np.concatenate([np.asarray(m[name]) for m in maps], axis=0)
            staged.append(self.jax.device_put(arr, sh))
        return staged

    def fresh_zeros(self):
        from jax.sharding import NamedSharding, PartitionSpec

        sh = NamedSharding(self.mesh, PartitionSpec("core"))
        return [
            self.jax.device_put(
                np.zeros((NCORES * z.shape[0], *z.shape[1:]), z.dtype), sh
            )
            for z in self.zero_outs
        ]

    def run(self, staged_inputs):
        outs = self.fn(*staged_inputs, *self.fresh_zeros())
        return self.unpack(outs)

    def unpack(self, outs):
        return [
            {
                name: np.asarray(outs[i]).reshape(
                    NCORES, *self.out_avals[i].shape
                )[c]
                for i, name in enumerate(self.out_names)
            }
            for c in range(NCORES)
        ]


_RUNNERS = {}


def _get_runner(flags):
    if flags not in _RUNNERS:
        _RUNNERS[flags] = _Runner(_get_nc(*flags))
    return _RUNNERS[flags]


def kernel(x, node_rotation_angles, Wq, bq, Wk, bk, Wv, bv, S):
    flags, maps = _in_maps(
        x, node_rotation_angles, Wq, bq, Wk, bk, Wv, bv, S
    )
    runner = _get_runner(flags)
    res = runner.run(runner.stage_inputs(maps))
    return _assemble(res)


def _burst(runner, staged, n):
    """Queue n executions without blocking in between; return wall time."""
    import time

    zsets = [runner.fresh_zeros() for _ in range(n)]
    for z in zsets:
        for a in z:
            a.block_until_ready()
    t0 = time.perf_counter()
    outs = None
    for z in zsets:
        outs = runner.fn(*staged, *z)
    for o in outs:
        o.block_until_ready()
    return time.perf_counter() - t0


def kernel_profiled(x, node_rotation_angles, Wq, bq, Wk, bk, Wv, bv, S,
                    n_lo=4, n_hi=16, reps=6):
    """kernel() + per-execution device time from the wall-clock slope of
    queued execution bursts (dispatch overhead cancels in the slope)."""
    flags, maps = _in_maps(
        x, node_rotation_angles, Wq, bq, Wk, bk, Wv, bv, S
    )
    runner = _get_runner(flags)
    staged = runner.stage_inputs(maps)
    res = runner.run(staged)  # warmup + compile
    lo, hi = [], []
    for _ in range(reps):
        lo.append(_burst(runner, staged, n_lo))
        hi.append(_burst(runner, staged, n_hi))
    ns = (min(hi) - min(lo)) / (n_hi - n_lo) * 1e9
    return _assemble(res), int(ns)


# revision 53
# speedup vs baseline: 11.4536x; 11.4536x over previous
"""Trainium2 Bass kernel for nn_MultiHeadDenseDotProductAttentionLayer.

Sharding: one attention head per NeuronCore (8 heads / 8 cores), per the
tensor-parallel hint.  Each core computes its head's Q/K projections from the
384-row slab of x that the reference's raw-view reshape maps to that head,
the V projection over all rows for its 64 weight columns, the [3072, 3072]
attention (scores computed transposed so the softmax denominator folds into
the A@V matmul as a ones-column), and writes its [3072, 64] output slice.

Host side only reshapes/slices/transposes numpy inputs to build per-core
input maps and concatenates the per-core output slices.
"""

import os
import sys

import numpy as np

for _p in ("/opt/trn_rl_repo", "/root/.axon_site/_ro/trn_rl_repo"):
    if os.path.isdir(_p) and _p not in sys.path:
        sys.path.insert(0, _p)

import concourse.bass as bass
import concourse.tile as tile
from concourse import bacc, mybir
from concourse.masks import make_identity

N = 3072
IN_DIM = 512
H = 8
D = 64
A = 8
HD = H * D          # 512
SLAB = N // H       # 384
NCORES = 8
KC = IN_DIM // 128  # 4 contraction chunks
RT = SLAB // 128    # 3 row tiles per slab
MT = N // 128       # 24 m-chunks
IT = N // 512       # 6 i-chunks
CLAMP_RAW = 40.0    # clip(score, 5) with score = raw/8  ->  min(raw, 40)
FP = mybir.dt.float32
FPR = mybir.dt.float32r


def _r(ap):
    """Reinterpret an fp32 AP as float32r: full-rate PE streaming for fp32
    data (vs 4 cycles/row for plain fp32 matmuls)."""
    return ap.bitcast(FPR)

# dtype for the clipped scores / exponentiated attention weights and the V
# operand of the second matmul.  float16 keeps ~5e-4 relative accuracy while
# enabling the 16-bit 2x packed read mode on the ACT exp pass.
E_DT = mybir.dt.float16
AF = mybir.ActivationFunctionType


def _build(has_bq, has_bk, has_bv):
    nc = bacc.Bacc()

    xT = nc.declare_dram_parameter("xT", [IN_DIM, N], E_DT, False)
    xsT = nc.declare_dram_parameter("xsT", [IN_DIM, SLAB], FPR, False)
    wq_d = nc.declare_dram_parameter("wq", [IN_DIM, HD], FPR, False)
    wk_d = nc.declare_dram_parameter("wk", [IN_DIM, HD], FPR, False)
    wv_d = nc.declare_dram_parameter("wv", [IN_DIM, D], E_DT, False)
    angT_d = nc.declare_dram_parameter("angT", [A, SLAB], FP, False)
    s_d = nc.declare_dram_parameter("S", [A, HD // 2], FP, False)
    if has_bq:
        bq_d = nc.declare_dram_parameter("bq", [1, HD], FP, False)
    if has_bk:
        bk_d = nc.declare_dram_parameter("bk", [1, HD], FP, False)
    if has_bv:
        bv_d = nc.declare_dram_parameter("bv", [1, D], FP, False)
    out_d = nc.declare_dram_parameter("out", [N, D], FP, True)

    with tile.TileContext(nc) as tc:
        with (
            tc.tile_pool(name="consts", bufs=1) as consts,
            tc.tile_pool(name="dram", bufs=1, space="DRAM") as dram,
        ):
            ident = consts.tile([128, 128], FP)
            make_identity(nc, ident)
            identr = consts.tile([128, 128], FPR)
            nc.vector.tensor_copy(identr, ident)
            halfpi = consts.tile([128, 1], FP)
            nc.vector.memset(halfpi, float(np.pi / 2))
            forty = consts.tile([128, 1], FP)
            nc.vector.memset(forty, CLAMP_RAW)
            five = consts.tile([128, 1], FP)
            nc.vector.memset(five, CLAMP_RAW / 8.0)

            # ---- constant loads (SP queue order == criticality) ------
            angT_sb = consts.tile([A, SLAB], FP)
            nc.sync.dma_start(out=angT_sb, in_=angT_d[:, :])
            s_sb = consts.tile([A, HD // 2], FP)
            nc.sync.dma_start(out=s_sb, in_=s_d[:, :])
            xsT_sb = consts.tile([128, KC, SLAB], FPR)
            nc.sync.dma_start(
                out=xsT_sb, in_=xsT.rearrange("(kc p) r -> p kc r", p=128)
            )
            wk_sb = consts.tile([128, KC, HD], FPR)
            nc.sync.dma_start(
                out=wk_sb, in_=wk_d.rearrange("(kc p) c -> p kc c", p=128)
            )
            wv_sb = consts.tile([128, KC, D], E_DT)
            nc.sync.dma_start(
                out=wv_sb, in_=wv_d.rearrange("(kc p) c -> p kc c", p=128)
            )
            wq_sb = consts.tile([128, KC, HD], FPR)
            nc.sync.dma_start(
                out=wq_sb, in_=wq_d.rearrange("(kc p) c -> p kc c", p=128)
            )
            if has_bq:
                bq_sb = consts.tile([1, HD], FP)
                nc.sync.dma_start(out=bq_sb, in_=bq_d[:, :])
            if has_bk:
                bk_sb = consts.tile([1, HD], FP)
                nc.sync.dma_start(out=bk_sb, in_=bk_d[:, :])
            if has_bv:
                bv_sb = consts.tile([1, D], FP)
                nc.sync.dma_start(out=bv_sb, in_=bv_d[:, :])
            if has_bq or has_bk or has_bv:
                ones_col = consts.tile([1, 128], FP)
                nc.vector.memset(ones_col, 1.0)

            # full x^T on the SWDGE queue (keeps the SP queue free for the
            # small latency-critical loads), split so the V projection can
            # start before the whole 6 MB lands
            xT_sb = consts.tile([128, KC, N], E_DT)
            for q in range(4):
                sl = slice(q * (N // 4), (q + 1) * (N // 4))
                nc.gpsimd.dma_start(
                    out=xT_sb[:, :, sl],
                    in_=xT[:, sl].rearrange("(kc p) m -> p kc m", p=128),
                )

            # persistent operands of the attention loop
            qdT = consts.tile([D, N], FPR)           # Q.reshape(3072,64).T
            ks_sb = consts.tile([D, N], FPR)         # K raw-view [64, 3072]
            v_sb = consts.tile([128, MT, D + 1], E_DT)  # [V | 1] per m-chunk
            k_scr = dram.tile([SLAB, HD], FPR)

            # ================= prologue ===============================
            with (
                tc.tile_pool(name="small", bufs=1) as small,
                tc.tile_pool(name="ppsum", bufs=2, space="PSUM") as ppsum,
                tc.tile_pool(name="trig", bufs=3) as trig,
                tc.tile_pool(name="qk", bufs=3) as qk,
            ):
                # softmax(S, axis=1) -> P, then column-doubled P_rep
                smax = small.tile([A, 1], FP)
                nc.vector.tensor_reduce(
                    out=smax, in_=s_sb, axis=mybir.AxisListType.X,
                    op=mybir.AluOpType.max,
                )
                negmax = small.tile([A, 1], FP)
                nc.vector.tensor_scalar_mul(negmax, smax, -1.0)
                p_sb = small.tile([A, HD // 2], FP)
                psum_acc = small.tile([A, 1], FP)
                nc.scalar.activation(
                    p_sb, s_sb, AF.Exp, bias=negmax, scale=1.0,
                    accum_out=psum_acc,
                )
                rec8 = small.tile([A, 1], FP)
                nc.vector.reciprocal(rec8, psum_acc)
                p2_sb = small.tile([A, HD // 2], FP)
                nc.vector.tensor_scalar_mul(p2_sb, p_sb, rec8)
                p_rep = small.tile([A, HD], FP)
                pr3 = p_rep.rearrange("a (c two) -> a c two", two=2)
                nc.scalar.copy(pr3[:, :, 0], p2_sb)
                nc.scalar.copy(pr3[:, :, 1], p2_sb)

                # rope combine helper: r = x*cos + shuffle(x)*sin_pm
                def rope(pr_ps, cos_t, spm, spm4):
                    r_t = qk.tile([128, HD], FPR, tag="rt", name="r_t")
                    nc.vector.tensor_tensor(
                        r_t, pr_ps, cos_t, mybir.AluOpType.mult
                    )
                    tmp = qk.tile([128, HD], FP, tag="tmp", name="tmp")
                    tmp4 = tmp.rearrange("p (cb h t) -> p cb h t", cb=8, h=2)
                    x4 = pr_ps.rearrange(
                        "p (cb t two) -> p cb t two", cb=8, two=2
                    )
                    nc.vector.tensor_tensor(
                        tmp4[:, :, 0, :], x4[:, :, :, 1], spm4[:, :, 0, :],
                        mybir.AluOpType.mult,
                    )
                    nc.vector.tensor_tensor(
                        tmp4[:, :, 1, :], x4[:, :, :, 0], spm4[:, :, 1, :],
                        mybir.AluOpType.mult,
                    )
                    nc.vector.tensor_tensor(r_t, r_t, tmp, mybir.AluOpType.add)
                    return r_t

                def proj(w_sb, b_sb, rsl):
                    pr_ps = ppsum.tile([128, HD], FP, tag="proj", name="pr", bufs=3)
                    if b_sb is not None:
                        nc.tensor.matmul(
                            pr_ps, ones_col, b_sb, start=True, stop=False
                        )
                    for kc in range(KC):
                        nc.tensor.matmul(
                            pr_ps,
                            xsT_sb[:, kc, rsl],
                            w_sb[:, kc, :],
                            start=(kc == 0 and b_sb is None),
                            stop=(kc == KC - 1),
                        )
                    return pr_ps

                # K pass first (plus theta -> cos/sin): the Ks round trip
                # gates the whole attention loop
                trigs = []
                for rt in range(RT):
                    rsl = slice(rt * 128, (rt + 1) * 128)
                    th_ps = ppsum.tile([128, HD], FP, tag="th")
                    nc.tensor.matmul(
                        th_ps, angT_sb[:, rsl], p_rep, start=True, stop=True
                    )
                    cos_t = trig.tile([128, HD], FP, tag="cos", name="cos_t")
                    nc.scalar.activation(cos_t, th_ps, AF.Sin, bias=halfpi)
                    # sin with the rotate-half sign pattern folded in:
                    # first 32 of each 64-block negative, last 32 positive
                    spm = trig.tile([128, HD], FP, tag="spm", name="spm")
                    spm4 = spm.rearrange("p (cb h t) -> p cb h t", cb=8, h=2)
                    thv = th_ps.rearrange("p (cb t) -> p cb t", cb=8)
                    nc.scalar.activation(
                        spm4[:, :, 0, :], thv[:, :, 0:32], AF.Sin, scale=-1.0
                    )
                    nc.scalar.activation(
                        spm4[:, :, 1, :], thv[:, :, 32:64], AF.Sin, scale=1.0
                    )
                    trigs.append((cos_t, spm, spm4))

                    pr_ps = proj(wk_sb, bk_sb if has_bk else None, rsl)
                    r_t = rope(pr_ps, cos_t, spm, spm4)
                    nc.sync.dma_start(out=k_scr[rsl, :], in_=r_t)

                # K raw view: row j of [64, 3072] = rows 6j..6j+6 of [384, 512]
                ks_v = k_scr.rearrange("(j rr) c -> j (rr c)", j=D)
                nc.sync.dma_start(
                    out=ks_sb[:, 0:N // 2], in_=ks_v[:, 0:N // 2]
                )
                nc.scalar.dma_start(
                    out=ks_sb[:, N // 2:N], in_=ks_v[:, N // 2:N]
                )

                # Q pass; qdT[j, rt*1024 + rr*8 + cb] = r_t[rr, 64cb + j]
                qdT_v = qdT.rearrange(
                    "j (rt rr cb) -> j rt rr cb", rt=RT, cb=8
                )
                for rt in range(RT):
                    rsl = slice(rt * 128, (rt + 1) * 128)
                    cos_t, spm, spm4 = trigs[rt]
                    pr_ps = proj(wq_sb, bq_sb if has_bq else None, rsl)
                    r_t = rope(pr_ps, cos_t, spm, spm4)
                    for cb in range(8):
                        tr_ps = ppsum.tile([D, 128], FPR, tag="tr")
                        nc.tensor.transpose(
                            tr_ps, r_t[:, cb * D:(cb + 1) * D], identr
                        )
                        nc.vector.tensor_copy(qdT_v[:, rt, :, cb], tr_ps)



            # ================= attention main loop ====================
            with (
                tc.tile_pool(name="opsum", bufs=1, space="PSUM") as opsum,
                tc.tile_pool(name="stpsum", bufs=2, space="PSUM") as stp,
                tc.tile_pool(name="ets", bufs=5) as ets,
                tc.tile_pool(name="fin", bufs=2) as fin,
                tc.tile_pool(name="outp", bufs=2) as outp,
            ):
                o_tiles = [
                    opsum.tile([D + 1, 512], FP, tag=f"o{i}", name=f"o{i}")
                    for i in range(IT)
                ]
                for mt in range(MT):
                    # V projection for this m-chunk (natural layout + ones
                    # column); interleaved here so prologue PE stays short
                    msl = slice(mt * 128, (mt + 1) * 128)
                    v_ps = stp.tile([128, D], FP, tag="st", name="v_ps")
                    if has_bv:
                        nc.tensor.matmul(
                            v_ps, ones_col, bv_sb, start=True, stop=False
                        )
                    for kc in range(KC):
                        nc.tensor.matmul(
                            v_ps,
                            xT_sb[:, kc, msl],
                            wv_sb[:, kc, :],
                            start=(kc == 0 and not has_bv),
                            stop=(kc == KC - 1),
                        )
                    nc.vector.tensor_copy(v_sb[:, mt, 0:D], v_ps)
                    nc.gpsimd.memset(v_sb[:, mt, D:D + 1], 1.0)

                    ks_l = ks_sb[:, mt * 128:(mt + 1) * 128]
                    v_l = v_sb[:, mt, :]
                    for it in range(IT):
                        st = stp.tile([128, 512], FP, tag="st")
                        nc.tensor.matmul(
                            st, ks_l, qdT[:, it * 512:(it + 1) * 512],
                            start=True, stop=True,
                        )
                        etc = ets.tile([128, 512], E_DT, tag="etc")
                        ete = ets.tile([128, 512], E_DT, tag="ete")
                        if (mt * IT + it) % 4 == 3:
                            # ACT-only clip+exp, offloading the DVE:
                            # exp(5 - relu(40-x)/8) == exp(min(x, 40)/8)
                            nc.scalar.activation(
                                etc, st, AF.Relu, bias=forty, scale=-1.0
                            )
                            nc.scalar.activation(
                                ete, etc, AF.Exp, bias=five, scale=-0.125
                            )
                        else:
                            nc.vector.tensor_scalar_min(etc, st, CLAMP_RAW)
                            nc.scalar.activation(ete, etc, AF.Exp, scale=0.125)
                        nc.tensor.matmul(
                            o_tiles[it], v_l, ete,
                            start=(mt == 0), stop=(mt == MT - 1),
                            skip_group_check=True,
                        )

                # normalize + transpose back to [i, d], write out
                for it in range(IT):
                    ot = fin.tile([D + 1, 512], FP, tag="ot")
                    nc.scalar.copy(ot, o_tiles[it])
                    ob = outp.tile([128, 4, D], FP, tag="ob")
                    for s in range(4):
                        on_ps = stp.tile([128, D + 1], FP, tag="st")
                        nc.tensor.transpose(
                            on_ps, ot[:, s * 128:(s + 1) * 128],
                            ident[0:D + 1, 0:D + 1],
                        )
                        recd = fin.tile([128, 1], FP, tag="recd")
                        nc.vector.reciprocal(recd, on_ps[:, D:D + 1])
                        nc.vector.tensor_scalar_mul(
                            ob[:, s, :], on_ps[:, 0:D], recd
                        )
                    nc.sync.dma_start(
                        out=out_d[it * 512:(it + 1) * 512, :].rearrange(
                            "(s p) d -> p s d", p=128
                        ),
                        in_=ob,
                    )

    nc.compile()
    nc.finalize()
    return nc


_CACHE = {}


def _get_nc(has_bq, has_bk, has_bv):
    key = (has_bq, has_bk, has_bv)
    if key not in _CACHE:
        _CACHE[key] = _build(*key)
    return _CACHE[key]


def _in_maps(x, node_rotation_angles, Wq, bq, Wk, bk, Wv, bv, S):
    f32 = np.float32
    x = np.asarray(x, f32)
    ang = np.asarray(node_rotation_angles, f32)
    Wq = np.asarray(Wq, f32)
    Wk = np.asarray(Wk, f32)
    Wv = np.asarray(Wv, f32)
    S = np.asarray(S, f32)
    bq = np.asarray(bq, f32)
    bk = np.asarray(bk, f32)
    bv = np.asarray(bv, f32)

    has_bq = bool(np.any(bq))
    has_bk = bool(np.any(bk))
    has_bv = bool(np.any(bv))

    xT = np.ascontiguousarray(x.T)
    xT16 = xT.astype(np.float16)
    angT = np.ascontiguousarray(ang.T)

    maps = []
    for h in range(NCORES):
        m = {
            "xT": xT16,
            "xsT": np.ascontiguousarray(xT[:, h * SLAB:(h + 1) * SLAB]),
            "wq": Wq,
            "wk": Wk,
            "wv": np.ascontiguousarray(
                Wv[:, h * D:(h + 1) * D]
            ).astype(np.float16),
            "angT": np.ascontiguousarray(angT[:, h * SLAB:(h + 1) * SLAB]),
            "S": S,
        }
        if has_bq:
            m["bq"] = bq.reshape(1, HD)
        if has_bk:
            m["bk"] = bk.reshape(1, HD)
        if has_bv:
            m["bv"] = np.ascontiguousarray(bv[h * D:(h + 1) * D]).reshape(1, D)
        maps.append(m)
    return (has_bq, has_bk, has_bv), maps


def _assemble(results):
    out = np.empty((N, HD), np.float32)
    for h in range(NCORES):
        out[:, h * D:(h + 1) * D] = results[h]["out"]
    return out.reshape(N, H, D)


class _Runner:
    """Persistent shard_map'd executor for the SPMD bass kernel.

    Mirrors bass2jax.run_bass_via_pjrt but keeps the compiled function and
    lets inputs stay on device across calls so execution can be timed
    without per-call host transfer / dispatch rebuild cost.
    """

    def __init__(self, nc):
        import jax
        from jax.sharding import Mesh, PartitionSpec
        from jax.experimental.shard_map import shard_map

        from concourse import bass2jax, mybir as _mb

        bass2jax.install_neuronx_cc_hook()
        self.nc = nc
        partition_name = (
            nc.partition_id_tensor.name if nc.partition_id_tensor else None
        )
        in_names, out_names, out_avals, zero_outs = [], [], [], []
        for alloc in nc.m.functions[0].allocations:
            if not isinstance(alloc, _mb.MemoryLocationSet):
                continue
            name = alloc.memorylocations[0].name
            if alloc.kind == "ExternalInput":
                if name != partition_name:
                    in_names.append(name)
            elif alloc.kind == "ExternalOutput":
                out_names.append(name)
                shape = tuple(alloc.tensor_shape)
                dtype = _mb.dt.np(alloc.dtype)
                out_avals.append(jax.core.ShapedArray(shape, dtype))
                zero_outs.append(np.zeros(shape, dtype))
        self.in_names = list(in_names)
        self.out_names = out_names
        self.out_avals = out_avals
        self.zero_outs = zero_outs
        n_params = len(in_names)
        all_names = in_names + out_names
        if partition_name is not None:
            all_names = all_names + [partition_name]

        def _body(*args):
            operands = list(args)
            if partition_name is not None:
                operands.append(bass2jax.partition_id_tensor())
            outs = bass2jax._bass_exec_p.bind(
                *operands,
                out_avals=tuple(out_avals),
                in_names=tuple(all_names),
                out_names=tuple(out_names),
                lowering_input_output_aliases=(),
                sim_require_finite=True,
                sim_require_nnan=True,
                nc=nc,
            )
            return tuple(outs)

        devices = jax.devices()[:NCORES]
        self.mesh = Mesh(np.asarray(devices), ("core",))
        n_outs = len(out_names)
        self.n_params = n_params
        self.n_outs = n_outs
        in_specs = (PartitionSpec("core"),) * (n_params + n_outs)
        out_specs = (PartitionSpec("core"),) * n_outs
        self.fn = jax.jit(
            shard_map(
                _body, mesh=self.mesh, in_specs=in_specs,
                out_specs=out_specs, check_rep=False,
            ),
            donate_argnums=tuple(range(n_params, n_params + n_outs)),
            keep_unused=True,
        )
        self._body = _body
        self._shard_map = shard_map
        self._PartitionSpec = PartitionSpec
        self.jax = jax

    def build_multi(self, k):
        """jit fn executing the kernel k times back-to-back on device.

        Takes (inputs..., zeros_0..., zeros_1..., ..., zeros_{k-1}...);
        bass effects keep the k custom calls ordered, so wall-time slope
        over k measures pure on-device execution time."""
        jax = self.jax
        np_, no, body = self.n_params, self.n_outs, self._body

        def _multi(*args):
            ins = args[:np_]
            outs = None
            for i in range(k):
                z = args[np_ + i * no: np_ + (i + 1) * no]
                outs = body(*ins, *z)
            return outs

        in_specs = (self._PartitionSpec("core"),) * (np_ + k * no)
        out_specs = (self._PartitionSpec("core"),) * no
        return jax.jit(
            self._shard_map(
                _multi, mesh=self.mesh, in_specs=in_specs,
                out_specs=out_specs, check_rep=False,
            ),
            donate_argnums=tuple(range(np_, np_ + k * no)),
            keep_unused=True,
        )

    def stage_inputs(self, maps):
        from jax.sharding import NamedSharding, PartitionSpec

        sh = NamedSharding(self.mesh, PartitionSpec("core"))
        staged = []
        for i, name in enumerate(self.in_names):
            arr = np.concatenate([np.asarray(m[name]) for m in maps], axis=0)
            staged.append(self.jax.device_put(arr, sh))
        return staged

    def fresh_zeros(self):
        from jax.sharding import NamedSharding, PartitionSpec

        sh = NamedSharding(self.mesh, PartitionSpec("core"))
        return [
            self.jax.device_put(
                np.zeros((NCORES * z.shape[0], *z.shape[1:]), z.dtype), sh
            )
            for z in self.zero_outs
        ]

    def run(self, staged_inputs):
        outs = self.fn(*staged_inputs, *self.fresh_zeros())
        return self.unpack(outs)

    def unpack(self, outs):
        return [
            {
                name: np.asarray(outs[i]).reshape(
                    NCORES, *self.out_avals[i].shape
                )[c]
                for i, name in enumerate(self.out_names)
            }
            for c in range(NCORES)
        ]


_RUNNERS = {}


def _get_runner(flags):
    if flags not in _RUNNERS:
        _RUNNERS[flags] = _Runner(_get_nc(*flags))
    return _RUNNERS[flags]


def kernel(x, node_rotation_angles, Wq, bq, Wk, bk, Wv, bv, S):
    flags, maps = _in_maps(
        x, node_rotation_angles, Wq, bq, Wk, bk, Wv, bv, S
    )
    runner = _get_runner(flags)
    res = runner.run(runner.stage_inputs(maps))
    return _assemble(res)


def _burst(runner, staged, n):
    """Queue n executions without blocking in between; return wall time."""
    import time

    zsets = [runner.fresh_zeros() for _ in range(n)]
    for z in zsets:
        for a in z:
            a.block_until_ready()
    t0 = time.perf_counter()
    outs = None
    for z in zsets:
        outs = runner.fn(*staged, *z)
    for o in outs:
        o.block_until_ready()
    return time.perf_counter() - t0


def kernel_profiled(x, node_rotation_angles, Wq, bq, Wk, bk, Wv, bv, S,
                    n_lo=4, n_hi=16, reps=6):
    """kernel() + per-execution device time from the wall-clock slope of
    queued execution bursts (dispatch overhead cancels in the slope)."""
    flags, maps = _in_maps(
        x, node_rotation_angles, Wq, bq, Wk, bk, Wv, bv, S
    )
    runner = _get_runner(flags)
    staged = runner.stage_inputs(maps)
    res = runner.run(staged)  # warmup + compile
    lo, hi = [], []
    for _ in range(reps):
        lo.append(_burst(runner, staged, n_lo))
        hi.append(_burst(runner, staged, n_hi))
    ns = (min(hi) - min(lo)) / (n_hi - n_lo) * 1e9
    return _assemble(res), int(ns)
